# revision 26
# baseline (speedup 1.0000x reference)
"""Gated-attention (Qwen-style) Trainium2 kernel — fp16, scheduling-optimized.

Sharding (8 cores): data-parallel over batch (2) x tensor-parallel over head
groups (4). Core c handles batch b=c//4 and head group g=c%4: q heads
4g..4g+3, kv heads 2g..2g+1, gate logits 4g..4g+3, w_o columns 512g..512g+512.
Each core computes a partial output y_g = attn_out_g @ w_o[:, cols_g].T in
fp16; the host sums the 4 partials per batch in f32.

Design (vs the 437us baseline, which was ACT-bound in phase 2 with PE HAM
clock oscillation; fp8 DoubleRow was evaluated and rejected — e4m3
quantization of any single matmul stage costs 3.6e-2+ rel err vs the 2e-2
gate):

Phase 1 (qkv projection + rope + v transpose + gate):
- Block 0 runs two stationary groups of 4 output tiles (halves the
  chunk-walk rate so startup DMA supply ~336GB/s keeps up); blocks 1-3 run
  four groups of 2 with the rope/v-processing of group g emitted after the
  matmuls of group g+1 (PE never stalls on PSUM->SBUF casts, which run on
  the otherwise-idle ACT).
- x is host-pre-tiled [NB*HID, IB] for sequential HBM reads; w_qkv is split
  into two column-halves so block-0's q-group weights land first.
- Gate logits ride a 128-wide stationary slice (cols 928:1056, gate in
  32-aligned rows 96..99) so FWL stays on; sigmod(gate) is broadcast to all
  128 partitions via a DRAM-bounce broadcast DMA into sgbc (no PE/ACT).

Phase 2 (attention + out-projection), software-pipelined j-pair loop:
- scores for j-pair jp are emitted one step ahead of the PV matmuls of
  jp-1; one [128,1024] exp (2 PSUM banks) per (head, j-pair) feeds fp16
  e-tiles; exp of the first pair writes the e-sum accumulator directly.
- e-sums accumulate on [128,1024] pairs (DVE 2x); one fold add feeds the
  denominator matmul, which uses an all-ones [128,128] stationary so den is
  broadcast to every PSUM row in one warm matmul (no 1-row/K=1 matmuls).
- The den chain after that matmul is pure DVE (recip, sigmoid-scale, OC
  write); PV PSUM drains to SBUF fp16 on ACT right after each j-loop so
  PSUM banks recycle and the chain runs off the PE critical path.
- Out-projection t-tiles of block i-1 are interleaved two-matmuls-per-slot
  into the j-loops as PE filler, emitted BEFORE each slot's PV matmuls so
  they execute during exp waits (covers exp latency, keeps HAM at K=8/8);
  remaining tiles slot between the deferred den chains.
- y output DMAs go per-o-slice on rotating gpsimd/sync/scalar queues so the
  final tile's 512KB store doesn't serialize on one DMA queue at the tail.

PSUM: scores 2x[128,1024] (4 banks) + psos 2 + shared den/psy 2 = 8.
Measured: 358-360us (fast power state), rel err 9.9e-4. PE busy ~94%
(~319us of a ~721k-cycle fp16 floor = 300us at 2.4GHz warm).
"""

import os
from contextlib import ExitStack

import numpy as np

B, S, HID = 2, 2048, 2048
NH, NKV, HD = 16, 8, 128
GATE = NH
KV_DIM = NKV * HD

N_CORES = 8
TPG = 4            # tensor-parallel group size (head groups)
QH = NH // TPG     # q heads per core = 4
KVH = NKV // TPG   # kv heads per core = 2
IB = 512           # phase-1 token block
NB = S // IB       # 4 blocks
JT = S // 128      # 16 key tiles
JP = JT // 2       # 8 key tile-pairs
IBLK = 512         # phase-2 query block
NI = S // IBLK     # 4 query blocks
WCOL = 1056        # packed qkv+gate weight cols (1028 used, padded to 32-align gate rows)
SCALE = 1.0 / float(np.sqrt(HD))

_CACHE = {}

LAST_EXEC_NS = None
LAST_RESULTS = None


def _build_program():
    import concourse.bass as bass
    import concourse.mybir as mybir
    from concourse import bacc
    from concourse.tile import TileContext

    F32 = mybir.dt.float32
    F32R = mybir.dt.float32r
    F16 = mybir.dt.float16
    AF = mybir.ActivationFunctionType

    nc = bacc.Bacc()

    xT_d = nc.dram_tensor("xT", [NB * HID, IB], F16, kind="ExternalInput")
    wqkv1_d = nc.dram_tensor("wqkv1", [HID, 512], F16, kind="ExternalInput")
    wqkv2_d = nc.dram_tensor("wqkv2", [HID, WCOL - 512], F16, kind="ExternalInput")
    woT_d = nc.dram_tensor("woT", [QH * HD, HID], F16, kind="ExternalInput")
    cosT_d = nc.dram_tensor("cosT", [HD, S], F16, kind="ExternalInput")
    sinT_d = nc.dram_tensor("sinT", [HD, S], F16, kind="ExternalInput")
    rotm_d = nc.dram_tensor("rotm", [HD, HD], F16, kind="ExternalInput")
    ident_d = nc.dram_tensor("ident", [128, 128], F16, kind="ExternalInput")
    onesr_d = nc.dram_tensor("onesr", [1, 128], F32R, kind="ExternalInput")
    onesN_d = nc.dram_tensor("onesN", [128, 128], F16, kind="ExternalInput")
    y_d = nc.dram_tensor("y", [S, HID], F16, kind="ExternalOutput")
    sgs_d = nc.dram_tensor("sgscratch", [QH, S], F16, kind="Internal")

    with TileContext(nc) as tc, ExitStack() as persist:
        const = persist.enter_context(tc.tile_pool(name="const", bufs=1))
        rotm_sb = const.tile([HD, HD], F16, tag="rotm", name="rotm")
        nc.scalar.dma_start(out=rotm_sb, in_=rotm_d[:, :])
        ident_sb = const.tile([128, 128], F16, tag="ident", name="ident")
        nc.scalar.dma_start(out=ident_sb, in_=ident_d[:, :])
        onesr_sb = const.tile([1, 128], F32R, tag="onesr", name="onesr")
        nc.scalar.dma_start(out=onesr_sb, in_=onesr_d[:, :])
        onesN_sb = const.tile([128, 128], F16, tag="onesN", name="onesN")
        nc.scalar.dma_start(out=onesN_sb, in_=onesN_d[:, :])

        # weights on ACT/DVE sequencers so x loads own the SP/Pool DGEs
        wpool = persist.enter_context(tc.tile_pool(name="w", bufs=1))
        wsb1 = [wpool.tile([128, 512], F16, tag=f"wa{h}", name=f"wa{h}") for h in range(16)]
        wsb2 = [wpool.tile([128, WCOL - 512], F16, tag=f"wb{h}", name=f"wb{h}")
                for h in range(16)]
        for h in range(16):
            nc.scalar.dma_start(out=wsb1[h], in_=wqkv1_d[128 * h:128 * (h + 1), :])
        for h in range(16):
            nc.scalar.dma_start(out=wsb2[h], in_=wqkv2_d[128 * h:128 * (h + 1), :])
        cos_sb = const.tile([HD, S], F16, tag="cos", name="cos")
        nc.scalar.dma_start(out=cos_sb, in_=cosT_d[:, :])
        sin_sb = const.tile([HD, S], F16, tag="sin", name="sin")
        nc.scalar.dma_start(out=sin_sb, in_=sinT_d[:, :])
        wopool = persist.enter_context(tc.tile_pool(name="wo", bufs=1))
        wo_sb = [wopool.tile([128, HID], F16, tag=f"wo{i}", name=f"wo{i}") for i in range(4)]

        qk_pool = persist.enter_context(tc.tile_pool(name="qk", bufs=1))
        qk_sb = [qk_pool.tile([128, S], F16, tag=f"qk{r}", name=f"qk{r}") for r in range(QH + KVH)]
        v_pool = persist.enter_context(tc.tile_pool(name="v", bufs=1))
        v_sb = [v_pool.tile([128, KVH * HD], F16, tag=f"v{t}", name=f"v{t}") for t in range(JT)]
        g_pool = persist.enter_context(tc.tile_pool(name="g", bufs=1))
        sgbc = [g_pool.tile([128, S], F16, tag=f"sg{h}", name=f"sg{h}") for h in range(QH)]

        # ---------------- phase 1: qkv projection + rope + v transpose -----
        with ExitStack() as ph1:
            xpool = ph1.enter_context(tc.tile_pool(name="x", bufs=32))
            tmppool = ph1.enter_context(tc.tile_pool(name="tmp", bufs=3))
            vrawpool = ph1.enter_context(tc.tile_pool(name="vraw", bufs=2))
            sgpool = ph1.enter_context(tc.tile_pool(name="sg", bufs=1))

            ps_acc = ph1.enter_context(tc.tile_pool(name="acc", bufs=4, space="PSUM"))
            ps_rot = ph1.enter_context(tc.tile_pool(name="rot", bufs=2, space="PSUM"))
            ps_tp = ph1.enter_context(tc.tile_pool(name="tp", bufs=2, space="PSUM"))

            pending_proc = [None]

            def emit_pending():
                if pending_proc[0] is not None:
                    pending_proc[0]()
                    pending_proc[0] = None

            def emit_gate(ib, xb, sl):
                psg_full = ps_rot.tile([128, IB], F32, tag="rot", name="psg")
                for h in range(16):
                    nc.tensor.matmul(psg_full, wsb2[h][:, WCOL - 640:WCOL - 512], xb[h],
                                     start=(h == 0), stop=(h == 15))
                emit_pending()
                psg = psg_full[96:96 + QH, :]
                eT = sgpool.tile([QH, IB], F32, tag="eT", name="eT")
                nc.scalar.activation(out=eT, in_=psg, func=AF.Exp, scale=-1.0)
                nc.vector.tensor_scalar_add(eT, eT, 1.0)
                sgT = sgpool.tile([QH, IB], F32, tag="sgT", name="sgT")
                nc.vector.reciprocal_approx_fast(out=sgT, in_=eT)
                sgT16 = sgpool.tile([QH, IB], F16, tag="sgT16", name="sgT16")
                nc.vector.tensor_copy(sgT16, sgT)
                nc.sync.dma_start(out=sgs_d[:, sl], in_=sgT16)
                for h in range(QH):
                    nc.scalar.dma_start(
                        out=sgbc[h][:, sl],
                        in_=sgs_d[h:h + 1, sl].to_broadcast((128, IB)))

            for ib in range(NB):
                sl = slice(IB * ib, IB * (ib + 1))
                xb = []
                for h in range(16):
                    xt = xpool.tile([128, IB], F16, tag="x", name="x")
                    eng = nc.gpsimd if h % 2 == 0 else nc.sync
                    eng.dma_start(
                        out=xt, in_=xT_d[HID * ib + 128 * h:HID * ib + 128 * (h + 1), :])
                    xb.append(xt)

                # stationary groups of output row-tiles. Block 0 uses two
                # groups of 4 (halves the chunk-walk rate so the startup
                # DMA supply keeps up); later blocks use 4 groups of 2
                # (software-pipelined procs).
                groups = ([[0, 1, 2, 3], [4, 5, 6, 7]] if ib == 0 else
                          [[0, 1], [2, 3], [4, 5], [6, 7]])
                for gi, rs in enumerate(groups):
                    accs = [ps_acc.tile([128, IB], F32, tag="acc", name="acc")
                            for _ in rs]
                    for h in range(16):
                        for r2, r in enumerate(rs):
                            wgrp = wsb1[h] if r < 4 else wsb2[h]
                            c0 = 128 * r if r < 4 else 128 * (r - 4)
                            nc.tensor.matmul(
                                accs[r2], wgrp[:, c0:c0 + 128], xb[h],
                                start=(h == 0), stop=(h == 15))

                    def make_proc(rs, accs, sl):
                        def proc():
                            for r2, r in enumerate(rs):
                                if r < QH + KVH:  # q or k row-tile: rope
                                    craw = tmppool.tile([128, IB], F16, tag="craw", name="craw")
                                    nc.scalar.copy(craw, accs[r2])
                                    rps = ps_rot.tile([128, IB], F32, tag="rot", name="rot")
                                    nc.tensor.matmul(rps, rotm_sb, craw, start=True, stop=True)
                                    t1 = tmppool.tile([128, IB], F32R, tag="t1", name="t1")
                                    nc.vector.tensor_mul(t1, accs[r2], cos_sb[:, sl])
                                    t2 = tmppool.tile([128, IB], F32R, tag="t2", name="t2")
                                    nc.vector.tensor_mul(t2, rps, sin_sb[:, sl])
                                    nc.vector.tensor_add(qk_sb[r][:, sl], t1, t2)
                                else:  # v row-tile: transpose to [tokens, d]
                                    vraw = vrawpool.tile([128, IB], F16, tag="vraw", name="vraw")
                                    nc.scalar.copy(vraw, accs[r2])
                                    vh = r - (QH + KVH)
                                    ibb = (sl.start // IB)
                                    for s2 in range(IB // 128):
                                        tp = ps_tp.tile([128, 128], F16, tag="tp", name="tp")
                                        nc.tensor.transpose(
                                            tp, vraw[:, 128 * s2:128 * (s2 + 1)], ident_sb)
                                        tt = (IB // 128) * ibb + s2
                                        nc.vector.tensor_copy(
                                            v_sb[tt][:, 128 * vh:128 * (vh + 1)], tp)
                        return proc

                    if ib == 0:
                        if gi == 0:
                            emit_gate(ib, xb, sl)
                        make_proc(rs, accs, sl)()
                    else:
                        emit_pending()
                        pending_proc[0] = make_proc(rs, accs, sl)

                if ib == 0:
                    continue
                emit_gate(ib, xb, sl)

            for cc in range(4):
                nc.gpsimd.dma_start(out=wo_sb[cc], in_=woT_d[128 * cc:128 * (cc + 1), :])
            emit_pending()

        # ---------------- phase 2: attention + gate + out-projection -------
        with ExitStack() as ph2:
            oc_pool = ph2.enter_context(tc.tile_pool(name="oc", bufs=1))
            OC = [oc_pool.tile([128, S], F16, tag=f"oc{h}", name=f"oc{h}") for h in range(QH)]
            epool = ph2.enter_context(tc.tile_pool(name="e", bufs=4))
            accpool = ph2.enter_context(tc.tile_pool(name="dacc", bufs=4))
            popool = ph2.enter_context(tc.tile_pool(name="po", bufs=4))
            scpool = ph2.enter_context(tc.tile_pool(name="sc", bufs=2))
            foldpool = ph2.enter_context(tc.tile_pool(name="fold", bufs=4))
            ypool = ph2.enter_context(tc.tile_pool(name="y", bufs=3))

            ps_s = ph2.enter_context(tc.tile_pool(name="pss", bufs=2, space="PSUM"))
            ps_o = ph2.enter_context(tc.tile_pool(name="pso", bufs=2, space="PSUM"))
            ps_sh = ph2.enter_context(tc.tile_pool(name="pssh", bufs=2, space="PSUM"))

            def oproj_steps(t, drain_all_dve):
                """out-projection for token tile t as 8 closures of ~2 MMs each."""
                state = {}

                def start():
                    state["ysb"] = ypool.tile([128, HID], F16, tag="y", name="y")

                steps = []
                for o in range(4):
                    def s_a(o=o):
                        if o == 0:
                            start()
                        state[o] = ps_sh.tile([128, IBLK], F32, tag="sh", name="psy")
                        for cc in range(2):
                            nc.tensor.matmul(
                                state[o], OC[cc][:, 128 * t:128 * (t + 1)],
                                wo_sb[cc][:, IBLK * o:IBLK * (o + 1)],
                                start=(cc == 0), stop=False)

                    def s_b(o=o):
                        for cc in range(2, 4):
                            nc.tensor.matmul(
                                state[o], OC[cc][:, 128 * t:128 * (t + 1)],
                                wo_sb[cc][:, IBLK * o:IBLK * (o + 1)],
                                start=False, stop=(cc == 3))
                        ysb = state["ysb"]
                        dst = ysb[:, IBLK * o:IBLK * (o + 1)]
                        if drain_all_dve:
                            nc.vector.tensor_copy(dst, state[o])
                        elif o % 2 == 0:
                            nc.scalar.copy(dst, state[o])
                        else:
                            nc.vector.tensor_copy(dst, state[o])
                        # per-o y DMA on alternating queues: starts output
                        # transfers early and spreads them off one DMA queue
                        eng = (nc.gpsimd, nc.sync, nc.scalar)[o % 3]
                        eng.dma_start(
                            out=y_d[128 * t:128 * (t + 1), IBLK * o:IBLK * (o + 1)],
                            in_=dst)

                    steps.append(s_a)
                    steps.append(s_b)
                return steps

            def emit_jloop(i, kv, fillers):
                """software-pipelined scores->exp->PV loop; fillers: list of
                closures (PE micro-steps) consumed one per pipeline slot."""
                isl = slice(IBLK * i, IBLK * (i + 1))
                psos = []
                accs2 = []
                for hh in range(2):
                    psos.append(ps_o.tile([128, IBLK], F32, tag="pso", name="pso"))
                    accs2.append(accpool.tile([128, 2 * IBLK], F16, tag="dacc", name="dacc"))
                fill_iter = iter(fillers)
                es_prev = None
                for jp in range(JP + 1):
                    es = None
                    if jp < JP:
                        es = []
                        for hh in range(2):
                            h = 2 * kv + hh
                            pss = ps_s.tile([128, 2 * IBLK], F32, tag="pss", name="pss")
                            for jj in range(2):
                                j = 2 * jp + jj
                                jsl = slice(128 * j, 128 * (j + 1))
                                nc.tensor.matmul(pss[:, IBLK * jj:IBLK * (jj + 1)],
                                                 qk_sb[QH + kv][:, jsl],
                                                 qk_sb[h][:, isl], start=True, stop=True)
                            if jp == 0:
                                e = accs2[hh]
                            else:
                                e = epool.tile([128, 2 * IBLK], F16, tag="e", name="e")
                            nc.scalar.activation(out=e, in_=pss, func=AF.Exp, scale=SCALE)
                            es.append(e)
                    if jp >= 1:
                        jq = jp - 1
                        # fillers first: they have no exp dependency, so they
                        # cover any exp latency before the PV matmuls
                        step = next(fill_iter, None)
                        if step is not None:
                            step()
                        for hh in range(2):
                            if jq > 0:
                                nc.vector.tensor_add(accs2[hh], accs2[hh], es_prev[hh])
                            for jj in range(2):
                                j = 2 * jq + jj
                                nc.tensor.matmul(psos[hh],
                                                 v_sb[j][:, 128 * kv:128 * (kv + 1)],
                                                 es_prev[hh][:, IBLK * jj:IBLK * (jj + 1)],
                                                 start=(j == 0), stop=(j == JT - 1))
                    es_prev = es
                # drain PV PSUM to SBUF fp16 (frees banks; decouples den chain)
                psout = []
                for hh in range(2):
                    po = popool.tile([128, IBLK], F16, tag="po", name="po")
                    nc.vector.tensor_copy(po, psos[hh])
                    psout.append(po)
                # fold the e-sum pairs early on DVE (feeds deferred den matmul)
                folds = []
                for hh in range(2):
                    fold = foldpool.tile([128, IBLK], F16, tag="fold", name="fold")
                    nc.vector.tensor_add(fold, accs2[hh][:, :IBLK], accs2[hh][:, IBLK:])
                    folds.append(fold)
                # leftover fillers (i==0 loops have none queued anyway)
                for step in fill_iter:
                    step()
                return psout, folds

            def emit_den_mms(kv, folds, dens):
                """den broadcast to all 128 rows via all-ones stationary."""
                for hh in range(2):
                    den = ps_sh.tile([128, IBLK], F32, tag="sh", name="den")
                    nc.tensor.matmul(den, onesN_sb, folds[hh], start=True, stop=True)
                    dens.append(den)

            def emit_sc_oc(i, kv, dens, psout):
                """recip + gate-scale + OC write, all DVE (no PE in chain)."""
                isl = slice(IBLK * i, IBLK * (i + 1))
                for hh in range(2):
                    h = 2 * kv + hh
                    rec = scpool.tile([128, IBLK], F32, tag="rec", name="rec")
                    nc.vector.reciprocal_approx_fast(out=rec, in_=dens[hh])
                    scg = scpool.tile([128, IBLK], F16, tag="scg", name="scg")
                    nc.vector.tensor_mul(scg, rec, sgbc[h][:, isl])
                    nc.vector.tensor_mul(OC[h][:, isl], psout[hh], scg)

            def emit_oproj_direct(t, cover=None):
                steps = oproj_steps(t, drain_all_dve=False)
                out = []
                for idx, step in enumerate(steps):
                    step()
                    if cover is not None and idx == 3:
                        cover()
                return out

            # emission schedule
            for i in range(NI):
                f0 = oproj_steps(4 * (i - 1) + 2, True) if i > 0 else []
                po0, fo0 = emit_jloop(i, 0, f0)
                f1 = oproj_steps(4 * (i - 1) + 3, True) if i > 0 else []
                po1, fo1 = emit_jloop(i, 1, f1)
                dens0 = []
                emit_den_mms(0, fo0, dens0)
                emit_sc_oc(i, 0, dens0, po0)
                if i > 0:
                    emit_oproj_direct(4 * (i - 1) + 0)
                dens1 = []
                emit_den_mms(1, fo1, dens1)
                emit_sc_oc(i, 1, dens1, po1)
                if i > 0:
                    emit_oproj_direct(4 * (i - 1) + 1)
            for t in (12, 13, 14, 15):
                emit_oproj_direct(t)

    nc.finalize()
    return nc


def kernel(hidden_states, cos, sin, w_qkv, w_o):
    global LAST_EXEC_NS, LAST_RESULTS
    from concourse.bass_utils import run_bass_kernel_spmd

    BF = np.float16
    hidden_states = np.asarray(hidden_states, dtype=np.float32)
    cos = np.asarray(cos, dtype=np.float32)
    sin = np.asarray(sin, dtype=np.float32)
    w_qkv = np.asarray(w_qkv, dtype=np.float32)
    w_o = np.asarray(w_o, dtype=np.float32)

    if "nc" not in _CACHE:
        _CACHE["nc"] = _build_program()
    nc = _CACHE["nc"]

    cosT = np.ascontiguousarray(cos.T).astype(BF)
    sinT = np.ascontiguousarray(sin.T).astype(BF)
    rotm = np.zeros((HD, HD), dtype=np.float32)
    for i in range(HD // 2):
        rotm[i + HD // 2, i] = -1.0   # rot[d'] = -q[d'+64] for d' < 64
        rotm[i, i + HD // 2] = 1.0    # rot[d'] = +q[d'-64] for d' >= 64
    rotm = rotm.astype(BF)
    ident = np.eye(128, dtype=np.float32).astype(BF)
    onesr = np.ones((1, 128), dtype=np.float32)
    onesN = np.ones((128, 128), dtype=np.float16)

    # pre-tiled x: [NB*HID, IB], block-major, sequential HBM reads per tile
    xT = []
    for b in range(B):
        xb_ = hidden_states[b].T.reshape(HID, NB, IB).transpose(1, 0, 2)
        xT.append(np.ascontiguousarray(xb_.reshape(NB * HID, IB)).astype(BF))
    in_maps = []
    for c in range(N_CORES):
        b, g = divmod(c, TPG)
        qr = w_qkv[512 * g:512 * (g + 1)]
        kr = w_qkv[HID + GATE + 256 * g:HID + GATE + 256 * (g + 1)]
        vr = w_qkv[HID + GATE + KV_DIM + 256 * g:HID + GATE + KV_DIM + 256 * (g + 1)]
        gr = w_qkv[HID + QH * g:HID + QH * (g + 1)]
        pad = np.zeros((WCOL - 1024 - QH, HID), dtype=np.float32)
        wqkvT = np.ascontiguousarray(
            np.concatenate([qr, kr, vr, gr, pad], axis=0).T).astype(BF)
        wqkv1 = np.ascontiguousarray(wqkvT[:, :512])
        wqkv2 = np.ascontiguousarray(wqkvT[:, 512:])
        woT = np.ascontiguousarray(w_o[:, 512 * g:512 * (g + 1)].T).astype(BF)
        in_maps.append({
            "xT": xT[b], "wqkv1": wqkv1, "wqkv2": wqkv2, "woT": woT,
            "cosT": cosT, "sinT": sinT, "rotm": rotm, "ident": ident,
            "onesr": onesr, "onesN": onesN,
        })

    trace = bool(int(os.environ.get("KERNEL_TRACE", "0")))
    out = run_bass_kernel_spmd(nc, in_maps, list(range(N_CORES)), trace=trace)
    LAST_EXEC_NS = out.exec_time_ns
    LAST_RESULTS = out
    y = np.zeros((B, S, HID), dtype=np.float32)
    for c in range(N_CORES):
        b = c // TPG
        y[b] += np.asarray(out.results[c]["y"]).astype(np.float32)
    return y


# revision 28
# speedup vs baseline: 1.1671x; 1.1671x over previous
"""Gated-attention (Qwen-style) Trainium2 kernel — fp16, scheduling-optimized.

Sharding (8 cores): data-parallel over batch (2) x tensor-parallel over head
groups (4). Core c handles batch b=c//4 and head group g=c%4: q heads
4g..4g+3, kv heads 2g..2g+1, gate logits 4g..4g+3, w_o columns 512g..512g+512.
Each core computes a partial output y_g = attn_out_g @ w_o[:, cols_g].T in
fp16; the host sums the 4 partials per batch in f32.

Design (vs the 437us baseline, which was ACT-bound in phase 2 with PE HAM
clock oscillation; fp8 DoubleRow was evaluated and rejected — e4m3
quantization of any single matmul stage costs 3.6e-2+ rel err vs the 2e-2
gate):

Phase 1 (qkv projection + rope + v transpose + gate):
- Block 0 runs two stationary groups of 4 output tiles (halves the
  chunk-walk rate so startup DMA supply ~336GB/s keeps up); blocks 1-3 run
  four groups of 2 with the rope/v-processing of group g emitted after the
  matmuls of group g+1 (PE never stalls on PSUM->SBUF casts, which run on
  the otherwise-idle ACT).
- x is host-pre-tiled [NB*HID, IB] for sequential HBM reads; w_qkv is split
  into two column-halves so block-0's q-group weights land first.
- Gate logits ride a 128-wide stationary slice (cols 928:1056, gate in
  32-aligned rows 96..99) so FWL stays on; sigmod(gate) is broadcast to all
  128 partitions via a DRAM-bounce broadcast DMA into sgbc (no PE/ACT).

Phase 2 (attention + out-projection), software-pipelined j-pair loop:
- scores for j-pair jp are emitted one step ahead of the PV matmuls of
  jp-1; one [128,1024] exp (2 PSUM banks) per (head, j-pair) feeds fp16
  e-tiles; exp of the first pair writes the e-sum accumulator directly.
- e-sums accumulate on [128,1024] pairs (DVE 2x); one fold add feeds the
  denominator matmul, which uses an all-ones [128,128] stationary so den is
  broadcast to every PSUM row in one warm matmul (no 1-row/K=1 matmuls).
- The den chain after that matmul is pure DVE (recip, sigmoid-scale, OC
  write); PV PSUM drains to SBUF fp16 on DVE right after each j-loop so
  PSUM banks recycle and the chain runs off the PE critical path (ACT is
  the j-loop pacer and keeps only exp + half the direct psy drains).
- Out-projection t-tiles of block i-1 are interleaved two-matmuls-per-slot
  into the j-loops as PE filler, emitted BEFORE each slot's PV matmuls so
  they execute during exp waits (covers exp latency, keeps HAM at K=8/8);
  remaining tiles slot between the deferred den chains.
- y output DMAs go per-o-slice on rotating gpsimd/sync/scalar queues so the
  final tile's 512KB store doesn't serialize on one DMA queue at the tail.

PSUM: scores 2x[128,1024] (4 banks) + psos 2 + shared den/psy 2 = 8.
Measured: ~356-360us (fast power state; the chip toggles to a 2.0GHz
state on some runs, scaling everything ~1.2x), rel err 9.9e-4. PE ~94% busy
(~319us of a ~721k-cycle fp16 floor = 300us at 2.4GHz warm).
"""

import os
from contextlib import ExitStack

import numpy as np

B, S, HID = 2, 2048, 2048
NH, NKV, HD = 16, 8, 128
GATE = NH
KV_DIM = NKV * HD

N_CORES = 8
TPG = 4            # tensor-parallel group size (head groups)
QH = NH // TPG     # q heads per core = 4
KVH = NKV // TPG   # kv heads per core = 2
IB = 512           # phase-1 token block
NB = S // IB       # 4 blocks
JT = S // 128      # 16 key tiles
JP = JT // 2       # 8 key tile-pairs
IBLK = 512         # phase-2 query block
NI = S // IBLK     # 4 query blocks
WCOL = 1056        # packed qkv+gate weight cols (1028 used, padded to 32-align gate rows)
SCALE = 1.0 / float(np.sqrt(HD))

_CACHE = {}

LAST_EXEC_NS = None
LAST_RESULTS = None


def _build_program():
    import concourse.bass as bass
    import concourse.mybir as mybir
    from concourse import bacc
    from concourse.tile import TileContext

    F32 = mybir.dt.float32
    F32R = mybir.dt.float32r
    F16 = mybir.dt.float16
    AF = mybir.ActivationFunctionType

    nc = bacc.Bacc()

    xT_d = nc.dram_tensor("xT", [NB * HID, IB], F16, kind="ExternalInput")
    wqkv1_d = nc.dram_tensor("wqkv1", [HID, 512], F16, kind="ExternalInput")
    wqkv2_d = nc.dram_tensor("wqkv2", [HID, WCOL - 512], F16, kind="ExternalInput")
    woT_d = nc.dram_tensor("woT", [QH * HD, HID], F16, kind="ExternalInput")
    cosT_d = nc.dram_tensor("cosT", [HD, S], F16, kind="ExternalInput")
    sinT_d = nc.dram_tensor("sinT", [HD, S], F16, kind="ExternalInput")
    rotm_d = nc.dram_tensor("rotm", [HD, HD], F16, kind="ExternalInput")
    ident_d = nc.dram_tensor("ident", [128, 128], F16, kind="ExternalInput")
    onesr_d = nc.dram_tensor("onesr", [1, 128], F32R, kind="ExternalInput")
    onesN_d = nc.dram_tensor("onesN", [128, 128], F16, kind="ExternalInput")
    y_d = nc.dram_tensor("y", [S, HID], F16, kind="ExternalOutput")
    sgs_d = nc.dram_tensor("sgscratch", [QH, S], F16, kind="Internal")

    with TileContext(nc) as tc, ExitStack() as persist:
        const = persist.enter_context(tc.tile_pool(name="const", bufs=1))
        rotm_sb = const.tile([HD, HD], F16, tag="rotm", name="rotm")
        nc.scalar.dma_start(out=rotm_sb, in_=rotm_d[:, :])
        ident_sb = const.tile([128, 128], F16, tag="ident", name="ident")
        nc.scalar.dma_start(out=ident_sb, in_=ident_d[:, :])
        onesr_sb = const.tile([1, 128], F32R, tag="onesr", name="onesr")
        nc.scalar.dma_start(out=onesr_sb, in_=onesr_d[:, :])
        onesN_sb = const.tile([128, 128], F16, tag="onesN", name="onesN")
        nc.scalar.dma_start(out=onesN_sb, in_=onesN_d[:, :])

        # weights on ACT/DVE sequencers so x loads own the SP/Pool DGEs
        wpool = persist.enter_context(tc.tile_pool(name="w", bufs=1))
        wsb1 = [wpool.tile([128, 512], F16, tag=f"wa{h}", name=f"wa{h}") for h in range(16)]
        wsb2 = [wpool.tile([128, WCOL - 512], F16, tag=f"wb{h}", name=f"wb{h}")
                for h in range(16)]
        for h in range(16):
            nc.scalar.dma_start(out=wsb1[h], in_=wqkv1_d[128 * h:128 * (h + 1), :])
        for h in range(16):
            nc.scalar.dma_start(out=wsb2[h], in_=wqkv2_d[128 * h:128 * (h + 1), :])
        cos_sb = const.tile([HD, S], F16, tag="cos", name="cos")
        nc.scalar.dma_start(out=cos_sb, in_=cosT_d[:, :])
        sin_sb = const.tile([HD, S], F16, tag="sin", name="sin")
        nc.scalar.dma_start(out=sin_sb, in_=sinT_d[:, :])
        wopool = persist.enter_context(tc.tile_pool(name="wo", bufs=1))
        wo_sb = [wopool.tile([128, HID], F16, tag=f"wo{i}", name=f"wo{i}") for i in range(4)]

        qk_pool = persist.enter_context(tc.tile_pool(name="qk", bufs=1))
        qk_sb = [qk_pool.tile([128, S], F16, tag=f"qk{r}", name=f"qk{r}") for r in range(QH + KVH)]
        v_pool = persist.enter_context(tc.tile_pool(name="v", bufs=1))
        v_sb = [v_pool.tile([128, KVH * HD], F16, tag=f"v{t}", name=f"v{t}") for t in range(JT)]
        g_pool = persist.enter_context(tc.tile_pool(name="g", bufs=1))
        sgbc = [g_pool.tile([128, S], F16, tag=f"sg{h}", name=f"sg{h}") for h in range(QH)]

        # ---------------- phase 1: qkv projection + rope + v transpose -----
        with ExitStack() as ph1:
            xpool = ph1.enter_context(tc.tile_pool(name="x", bufs=32))
            tmppool = ph1.enter_context(tc.tile_pool(name="tmp", bufs=3))
            vrawpool = ph1.enter_context(tc.tile_pool(name="vraw", bufs=2))
            sgpool = ph1.enter_context(tc.tile_pool(name="sg", bufs=1))

            ps_acc = ph1.enter_context(tc.tile_pool(name="acc", bufs=4, space="PSUM"))
            ps_rot = ph1.enter_context(tc.tile_pool(name="rot", bufs=2, space="PSUM"))
            ps_tp = ph1.enter_context(tc.tile_pool(name="tp", bufs=2, space="PSUM"))

            pending_proc = [None]

            def emit_pending():
                if pending_proc[0] is not None:
                    pending_proc[0]()
                    pending_proc[0] = None

            def emit_gate(ib, xb, sl):
                psg_full = ps_rot.tile([128, IB], F32, tag="rot", name="psg")
                for h in range(16):
                    nc.tensor.matmul(psg_full, wsb2[h][:, WCOL - 640:WCOL - 512], xb[h],
                                     start=(h == 0), stop=(h == 15))
                emit_pending()
                psg = psg_full[96:96 + QH, :]
                eT = sgpool.tile([QH, IB], F32, tag="eT", name="eT")
                nc.scalar.activation(out=eT, in_=psg, func=AF.Exp, scale=-1.0)
                nc.vector.tensor_scalar_add(eT, eT, 1.0)
                sgT = sgpool.tile([QH, IB], F32, tag="sgT", name="sgT")
                nc.vector.reciprocal_approx_fast(out=sgT, in_=eT)
                sgT16 = sgpool.tile([QH, IB], F16, tag="sgT16", name="sgT16")
                nc.vector.tensor_copy(sgT16, sgT)
                nc.sync.dma_start(out=sgs_d[:, sl], in_=sgT16)
                for h in range(QH):
                    nc.scalar.dma_start(
                        out=sgbc[h][:, sl],
                        in_=sgs_d[h:h + 1, sl].to_broadcast((128, IB)))

            for ib in range(NB):
                sl = slice(IB * ib, IB * (ib + 1))
                xb = []
                for h in range(16):
                    xt = xpool.tile([128, IB], F16, tag="x", name="x")
                    # block 0 is latency-critical: use sync's hardware DGE for
                    # every tile (gpsimd's software DGE costs ~640ns/descriptor,
                    # staggering even-h tiles at startup). Later blocks
                    # prefetch with slack and spread across both engines.
                    if ib == 0:
                        eng = nc.sync
                    else:
                        eng = nc.gpsimd if h % 2 == 0 else nc.sync
                    eng.dma_start(
                        out=xt, in_=xT_d[HID * ib + 128 * h:HID * ib + 128 * (h + 1), :])
                    xb.append(xt)

                # stationary groups of output row-tiles. Block 0 uses two
                # groups of 4 (halves the chunk-walk rate so the startup
                # DMA supply keeps up); later blocks use 4 groups of 2
                # (software-pipelined procs).
                groups = ([[0, 1, 2, 3], [4, 5, 6, 7]] if ib == 0 else
                          [[0, 1], [2, 3], [4, 5], [6, 7]])
                for gi, rs in enumerate(groups):
                    accs = [ps_acc.tile([128, IB], F32, tag="acc", name="acc")
                            for _ in rs]
                    for h in range(16):
                        for r2, r in enumerate(rs):
                            wgrp = wsb1[h] if r < 4 else wsb2[h]
                            c0 = 128 * r if r < 4 else 128 * (r - 4)
                            nc.tensor.matmul(
                                accs[r2], wgrp[:, c0:c0 + 128], xb[h],
                                start=(h == 0), stop=(h == 15))

                    def make_proc(rs, accs, sl):
                        def proc():
                            for r2, r in enumerate(rs):
                                if r < QH + KVH:  # q or k row-tile: rope
                                    craw = tmppool.tile([128, IB], F16, tag="craw", name="craw")
                                    nc.scalar.copy(craw, accs[r2])
                                    rps = ps_rot.tile([128, IB], F32, tag="rot", name="rot")
                                    nc.tensor.matmul(rps, rotm_sb, craw, start=True, stop=True)
                                    t1 = tmppool.tile([128, IB], F32R, tag="t1", name="t1")
                                    nc.vector.tensor_mul(t1, accs[r2], cos_sb[:, sl])
                                    t2 = tmppool.tile([128, IB], F32R, tag="t2", name="t2")
                                    nc.vector.tensor_mul(t2, rps, sin_sb[:, sl])
                                    nc.vector.tensor_add(qk_sb[r][:, sl], t1, t2)
                                else:  # v row-tile: transpose to [tokens, d]
                                    vraw = vrawpool.tile([128, IB], F16, tag="vraw", name="vraw")
                                    nc.scalar.copy(vraw, accs[r2])
                                    vh = r - (QH + KVH)
                                    ibb = (sl.start // IB)
                                    for s2 in range(IB // 128):
                                        tp = ps_tp.tile([128, 128], F16, tag="tp", name="tp")
                                        nc.tensor.transpose(
                                            tp, vraw[:, 128 * s2:128 * (s2 + 1)], ident_sb)
                                        tt = (IB // 128) * ibb + s2
                                        nc.vector.tensor_copy(
                                            v_sb[tt][:, 128 * vh:128 * (vh + 1)], tp)
                        return proc

                    if ib == 0:
                        if gi == 0:
                            emit_gate(ib, xb, sl)
                        make_proc(rs, accs, sl)()
                    else:
                        emit_pending()
                        pending_proc[0] = make_proc(rs, accs, sl)

                if ib == 0:
                    continue
                emit_gate(ib, xb, sl)

            for cc in range(4):
                nc.gpsimd.dma_start(out=wo_sb[cc], in_=woT_d[128 * cc:128 * (cc + 1), :])
            emit_pending()

        # ---------------- phase 2: attention + gate + out-projection -------
        with ExitStack() as ph2:
            oc_pool = ph2.enter_context(tc.tile_pool(name="oc", bufs=1))
            OC = [oc_pool.tile([128, S], F16, tag=f"oc{h}", name=f"oc{h}") for h in range(QH)]
            epool = ph2.enter_context(tc.tile_pool(name="e", bufs=4))
            accpool = ph2.enter_context(tc.tile_pool(name="dacc", bufs=4))
            popool = ph2.enter_context(tc.tile_pool(name="po", bufs=4))
            scpool = ph2.enter_context(tc.tile_pool(name="sc", bufs=2))
            foldpool = ph2.enter_context(tc.tile_pool(name="fold", bufs=4))
            ypool = ph2.enter_context(tc.tile_pool(name="y", bufs=3))

            ps_s = ph2.enter_context(tc.tile_pool(name="pss", bufs=2, space="PSUM"))
            ps_o = ph2.enter_context(tc.tile_pool(name="pso", bufs=2, space="PSUM"))
            ps_sh = ph2.enter_context(tc.tile_pool(name="pssh", bufs=2, space="PSUM"))

            def oproj_steps(t, drain_all_dve):
                """out-projection for token tile t as 8 closures of ~2 MMs each."""
                state = {}

                def start():
                    state["ysb"] = ypool.tile([128, HID], F16, tag="y", name="y")

                steps = []
                for o in range(4):
                    def s_a(o=o):
                        if o == 0:
                            start()
                        state[o] = ps_sh.tile([128, IBLK], F32, tag="sh", name="psy")
                        for cc in range(2):
                            nc.tensor.matmul(
                                state[o], OC[cc][:, 128 * t:128 * (t + 1)],
                                wo_sb[cc][:, IBLK * o:IBLK * (o + 1)],
                                start=(cc == 0), stop=False)

                    def s_b(o=o):
                        for cc in range(2, 4):
                            nc.tensor.matmul(
                                state[o], OC[cc][:, 128 * t:128 * (t + 1)],
                                wo_sb[cc][:, IBLK * o:IBLK * (o + 1)],
                                start=False, stop=(cc == 3))
                        ysb = state["ysb"]
                        dst = ysb[:, IBLK * o:IBLK * (o + 1)]
                        if drain_all_dve:
                            nc.vector.tensor_copy(dst, state[o])
                        elif o % 2 == 0:
                            nc.scalar.copy(dst, state[o])
                        else:
                            nc.vector.tensor_copy(dst, state[o])
                        # per-o y DMA on alternating queues: starts output
                        # transfers early and spreads them off one DMA queue
                        eng = (nc.gpsimd, nc.sync, nc.scalar)[o % 3]
                        eng.dma_start(
                            out=y_d[128 * t:128 * (t + 1), IBLK * o:IBLK * (o + 1)],
                            in_=dst)

                    steps.append(s_a)
                    steps.append(s_b)
                return steps

            def emit_jloop(i, kv, fillers):
                """software-pipelined scores->exp->PV loop; fillers: list of
                closures (PE micro-steps) consumed one per pipeline slot."""
                isl = slice(IBLK * i, IBLK * (i + 1))
                psos = []
                accs2 = []
                for hh in range(2):
                    psos.append(ps_o.tile([128, IBLK], F32, tag="pso", name="pso"))
                    accs2.append(accpool.tile([128, 2 * IBLK], F16, tag="dacc", name="dacc"))
                fill_iter = iter(fillers)
                es_prev = None
                for jp in range(JP + 1):
                    es = None
                    if jp < JP:
                        es = []
                        for hh in range(2):
                            h = 2 * kv + hh
                            pss = ps_s.tile([128, 2 * IBLK], F32, tag="pss", name="pss")
                            for jj in range(2):
                                j = 2 * jp + jj
                                jsl = slice(128 * j, 128 * (j + 1))
                                nc.tensor.matmul(pss[:, IBLK * jj:IBLK * (jj + 1)],
                                                 qk_sb[QH + kv][:, jsl],
                                                 qk_sb[h][:, isl], start=True, stop=True)
                            if jp == 0:
                                e = accs2[hh]
                            else:
                                e = epool.tile([128, 2 * IBLK], F16, tag="e", name="e")
                            nc.scalar.activation(out=e, in_=pss, func=AF.Exp, scale=SCALE)
                            es.append(e)
                    if jp >= 1:
                        jq = jp - 1
                        # fillers first: they have no exp dependency, so they
                        # cover any exp latency before the PV matmuls
                        step = next(fill_iter, None)
                        if step is not None:
                            step()
                        for hh in range(2):
                            if jq > 0:
                                nc.vector.tensor_add(accs2[hh], accs2[hh], es_prev[hh])
                            for jj in range(2):
                                j = 2 * jq + jj
                                nc.tensor.matmul(psos[hh],
                                                 v_sb[j][:, 128 * kv:128 * (kv + 1)],
                                                 es_prev[hh][:, IBLK * jj:IBLK * (jj + 1)],
                                                 start=(j == 0), stop=(j == JT - 1))
                    es_prev = es
                # drain PV PSUM to SBUF fp16 (frees banks; decouples den chain)
                psout = []
                for hh in range(2):
                    po = popool.tile([128, IBLK], F16, tag="po", name="po")
                    nc.vector.tensor_copy(po, psos[hh])
                    psout.append(po)
                # fold the e-sum pairs early on DVE (feeds deferred den matmul)
                folds = []
                for hh in range(2):
                    fold = foldpool.tile([128, IBLK], F16, tag="fold", name="fold")
                    nc.vector.tensor_add(fold, accs2[hh][:, :IBLK], accs2[hh][:, IBLK:])
                    folds.append(fold)
                # leftover fillers (i==0 loops have none queued anyway)
                for step in fill_iter:
                    step()
                return psout, folds

            def emit_den_mms(kv, folds, dens):
                """den broadcast to all 128 rows via all-ones stationary."""
                for hh in range(2):
                    den = ps_sh.tile([128, IBLK], F32, tag="sh", name="den")
                    nc.tensor.matmul(den, onesN_sb, folds[hh], start=True, stop=True)
                    dens.append(den)

            def emit_sc_oc(i, kv, dens, psout):
                """recip + gate-scale + OC write, all DVE (no PE in chain)."""
                isl = slice(IBLK * i, IBLK * (i + 1))
                for hh in range(2):
                    h = 2 * kv + hh
                    rec = scpool.tile([128, IBLK], F32, tag="rec", name="rec")
                    nc.vector.reciprocal_approx_fast(out=rec, in_=dens[hh])
                    scg = scpool.tile([128, IBLK], F16, tag="scg", name="scg")
                    nc.vector.tensor_mul(scg, rec, sgbc[h][:, isl])
                    nc.vector.tensor_mul(OC[h][:, isl], psout[hh], scg)

            def emit_oproj_direct(t, cover=None):
                steps = oproj_steps(t, drain_all_dve=False)
                out = []
                for idx, step in enumerate(steps):
                    step()
                    if cover is not None and idx == 3:
                        cover()
                return out

            # emission schedule
            for i in range(NI):
                f0 = oproj_steps(4 * (i - 1) + 2, True) if i > 0 else []
                po0, fo0 = emit_jloop(i, 0, f0)
                f1 = oproj_steps(4 * (i - 1) + 3, True) if i > 0 else []
                po1, fo1 = emit_jloop(i, 1, f1)
                dens0 = []
                emit_den_mms(0, fo0, dens0)
                emit_sc_oc(i, 0, dens0, po0)
                if i > 0:
                    emit_oproj_direct(4 * (i - 1) + 0)
                dens1 = []
                emit_den_mms(1, fo1, dens1)
                emit_sc_oc(i, 1, dens1, po1)
                if i > 0:
                    emit_oproj_direct(4 * (i - 1) + 1)
            for t in (12, 13, 14, 15):
                emit_oproj_direct(t)

    nc.finalize()
    return nc


def kernel(hidden_states, cos, sin, w_qkv, w_o):
    global LAST_EXEC_NS, LAST_RESULTS
    from concourse.bass_utils import run_bass_kernel_spmd

    BF = np.float16
    hidden_states = np.asarray(hidden_states, dtype=np.float32)
    cos = np.asarray(cos, dtype=np.float32)
    sin = np.asarray(sin, dtype=np.float32)
    w_qkv = np.asarray(w_qkv, dtype=np.float32)
    w_o = np.asarray(w_o, dtype=np.float32)

    if "nc" not in _CACHE:
        _CACHE["nc"] = _build_program()
    nc = _CACHE["nc"]

    cosT = np.ascontiguousarray(cos.T).astype(BF)
    sinT = np.ascontiguousarray(sin.T).astype(BF)
    rotm = np.zeros((HD, HD), dtype=np.float32)
    for i in range(HD // 2):
        rotm[i + HD // 2, i] = -1.0   # rot[d'] = -q[d'+64] for d' < 64
        rotm[i, i + HD // 2] = 1.0    # rot[d'] = +q[d'-64] for d' >= 64
    rotm = rotm.astype(BF)
    ident = np.eye(128, dtype=np.float32).astype(BF)
    onesr = np.ones((1, 128), dtype=np.float32)
    onesN = np.ones((128, 128), dtype=np.float16)

    # pre-tiled x: [NB*HID, IB], block-major, sequential HBM reads per tile
    xT = []
    for b in range(B):
        xb_ = hidden_states[b].T.reshape(HID, NB, IB).transpose(1, 0, 2)
        xT.append(np.ascontiguousarray(xb_.reshape(NB * HID, IB)).astype(BF))
    in_maps = []
    for c in range(N_CORES):
        b, g = divmod(c, TPG)
        qr = w_qkv[512 * g:512 * (g + 1)]
        kr = w_qkv[HID + GATE + 256 * g:HID + GATE + 256 * (g + 1)]
        vr = w_qkv[HID + GATE + KV_DIM + 256 * g:HID + GATE + KV_DIM + 256 * (g + 1)]
        gr = w_qkv[HID + QH * g:HID + QH * (g + 1)]
        pad = np.zeros((WCOL - 1024 - QH, HID), dtype=np.float32)
        wqkvT = np.ascontiguousarray(
            np.concatenate([qr, kr, vr, gr, pad], axis=0).T).astype(BF)
        wqkv1 = np.ascontiguousarray(wqkvT[:, :512])
        wqkv2 = np.ascontiguousarray(wqkvT[:, 512:])
        woT = np.ascontiguousarray(w_o[:, 512 * g:512 * (g + 1)].T).astype(BF)
        in_maps.append({
            "xT": xT[b], "wqkv1": wqkv1, "wqkv2": wqkv2, "woT": woT,
            "cosT": cosT, "sinT": sinT, "rotm": rotm, "ident": ident,
            "onesr": onesr, "onesN": onesN,
        })

    trace = bool(int(os.environ.get("KERNEL_TRACE", "0")))
    out = run_bass_kernel_spmd(nc, in_maps, list(range(N_CORES)), trace=trace)
    LAST_EXEC_NS = out.exec_time_ns
    LAST_RESULTS = out
    y = np.zeros((B, S, HID), dtype=np.float32)
    for c in range(N_CORES):
        b = c // TPG
        y[b] += np.asarray(out.results[c]["y"]).astype(np.float32)
    return y


# revision 29
# speedup vs baseline: 1.1809x; 1.0118x over previous
"""Gated-attention (Qwen-style) Trainium2 kernel — fp16, scheduling-optimized.

Sharding (8 cores): data-parallel over batch (2) x tensor-parallel over head
groups (4). Core c handles batch b=c//4 and head group g=c%4: q heads
4g..4g+3, kv heads 2g..2g+1, gate logits 4g..4g+3, w_o columns 512g..512g+512.
Each core computes a partial output y_g = attn_out_g @ w_o[:, cols_g].T in
fp16; the host sums the 4 partials per batch in f32.

Design (vs the 437us baseline, which was ACT-bound in phase 2 with PE HAM
clock oscillation; fp8 DoubleRow was evaluated and rejected — e4m3
quantization of any single matmul stage costs 3.6e-2+ rel err vs the 2e-2
gate):

Phase 1 (qkv projection + rope + v transpose + gate):
- Block 0 runs two stationary groups of 4 output tiles (halves the
  chunk-walk rate so startup DMA supply ~336GB/s keeps up); blocks 1-3 run
  four groups of 2 with the rope/v-processing of group g emitted after the
  matmuls of group g+1 (PE never stalls on PSUM->SBUF casts, which run on
  the otherwise-idle ACT).
- x is host-pre-tiled [NB*HID, IB] for sequential HBM reads; w_qkv is split
  into two column-halves so block-0's q-group weights land first.
- Gate logits ride a 128-wide stationary slice (cols 928:1056, gate in
  32-aligned rows 96..99) so FWL stays on; sigmod(gate) is broadcast to all
  128 partitions via a DRAM-bounce broadcast DMA into sgbc (no PE/ACT).

Phase 2 (attention + out-projection), software-pipelined j-pair loop:
- scores for j-pair jp are emitted one step ahead of the PV matmuls of
  jp-1; one [128,1024] exp (2 PSUM banks) per (head, j-pair) feeds fp16
  e-tiles; exp of the first pair writes the e-sum accumulator directly.
- e-sums accumulate on [128,1024] pairs (DVE 2x); one fold add feeds the
  denominator matmul, which uses an all-ones [128,128] stationary so den is
  broadcast to every PSUM row in one warm matmul (no 1-row/K=1 matmuls).
- The den chain after that matmul is pure DVE (recip, sigmoid-scale, OC
  write); PV PSUM drains to SBUF fp16 on DVE right after each j-loop so
  PSUM banks recycle and the chain runs off the PE critical path (ACT is
  the j-loop pacer and keeps only exp + half the direct psy drains).
- Out-projection t-tiles of block i-1 are interleaved two-matmuls-per-slot
  into the j-loops as PE filler, emitted BEFORE each slot's PV matmuls so
  they execute during exp waits (covers exp latency, keeps HAM at K=8/8);
  remaining tiles slot between the deferred den chains.
- y output DMAs go per-o-slice on rotating gpsimd/sync/scalar queues so the
  final tile's 512KB store doesn't serialize on one DMA queue at the tail.

PSUM: scores 2x[128,1024] (4 banks) + psos 2 + shared den/psy 2 = 8.
Measured: ~356-360us (fast power state; the chip toggles to a 2.0GHz
state on some runs, scaling everything ~1.2x), rel err 9.9e-4. PE ~94% busy
(~319us of a ~721k-cycle fp16 floor = 300us at 2.4GHz warm).
"""

import os
from contextlib import ExitStack

import numpy as np

B, S, HID = 2, 2048, 2048
NH, NKV, HD = 16, 8, 128
GATE = NH
KV_DIM = NKV * HD

N_CORES = 8
TPG = 4            # tensor-parallel group size (head groups)
QH = NH // TPG     # q heads per core = 4
KVH = NKV // TPG   # kv heads per core = 2
IB = 512           # phase-1 token block
NB = S // IB       # 4 blocks
JT = S // 128      # 16 key tiles
JP = JT // 2       # 8 key tile-pairs
IBLK = 512         # phase-2 query block
NI = S // IBLK     # 4 query blocks
WCOL = 1056        # packed qkv+gate weight cols (1028 used, padded to 32-align gate rows)
SCALE = 1.0 / float(np.sqrt(HD))

_CACHE = {}

LAST_EXEC_NS = None
LAST_RESULTS = None


def _build_program():
    import concourse.bass as bass
    import concourse.mybir as mybir
    from concourse import bacc
    from concourse.tile import TileContext

    F32 = mybir.dt.float32
    F32R = mybir.dt.float32r
    F16 = mybir.dt.float16
    AF = mybir.ActivationFunctionType

    nc = bacc.Bacc()

    xT_d = nc.dram_tensor("xT", [NB * HID, IB], F16, kind="ExternalInput")
    wqkv1_d = nc.dram_tensor("wqkv1", [HID, 512], F16, kind="ExternalInput")
    wqkv2_d = nc.dram_tensor("wqkv2", [HID, WCOL - 512], F16, kind="ExternalInput")
    woT_d = nc.dram_tensor("woT", [QH * HD, HID], F16, kind="ExternalInput")
    cosT_d = nc.dram_tensor("cosT", [HD, S], F16, kind="ExternalInput")
    sinT_d = nc.dram_tensor("sinT", [HD, S], F16, kind="ExternalInput")
    rotm_d = nc.dram_tensor("rotm", [HD, HD], F16, kind="ExternalInput")
    ident_d = nc.dram_tensor("ident", [128, 128], F16, kind="ExternalInput")
    onesr_d = nc.dram_tensor("onesr", [1, 128], F32R, kind="ExternalInput")
    onesN_d = nc.dram_tensor("onesN", [128, 128], F16, kind="ExternalInput")
    y_d = nc.dram_tensor("y", [S, HID], F16, kind="ExternalOutput")
    sgs_d = nc.dram_tensor("sgscratch", [QH, S], F16, kind="Internal")

    with TileContext(nc) as tc, ExitStack() as persist:
        const = persist.enter_context(tc.tile_pool(name="const", bufs=1))
        rotm_sb = const.tile([HD, HD], F16, tag="rotm", name="rotm")
        nc.scalar.dma_start(out=rotm_sb, in_=rotm_d[:, :])
        ident_sb = const.tile([128, 128], F16, tag="ident", name="ident")
        nc.scalar.dma_start(out=ident_sb, in_=ident_d[:, :])
        onesr_sb = const.tile([1, 128], F32R, tag="onesr", name="onesr")
        nc.scalar.dma_start(out=onesr_sb, in_=onesr_d[:, :])
        onesN_sb = const.tile([128, 128], F16, tag="onesN", name="onesN")
        nc.scalar.dma_start(out=onesN_sb, in_=onesN_d[:, :])

        # weights on ACT/DVE sequencers so x loads own the SP/Pool DGEs
        wpool = persist.enter_context(tc.tile_pool(name="w", bufs=1))
        wsb1 = [wpool.tile([128, 512], F16, tag=f"wa{h}", name=f"wa{h}") for h in range(16)]
        wsb2 = [wpool.tile([128, WCOL - 512], F16, tag=f"wb{h}", name=f"wb{h}")
                for h in range(16)]
        for h in range(16):
            nc.scalar.dma_start(out=wsb1[h], in_=wqkv1_d[128 * h:128 * (h + 1), :])
        for h in range(16):
            nc.scalar.dma_start(out=wsb2[h], in_=wqkv2_d[128 * h:128 * (h + 1), :])
        cos_sb = const.tile([HD, S], F16, tag="cos", name="cos")
        nc.scalar.dma_start(out=cos_sb, in_=cosT_d[:, :])
        sin_sb = const.tile([HD, S], F16, tag="sin", name="sin")
        nc.scalar.dma_start(out=sin_sb, in_=sinT_d[:, :])
        wopool = persist.enter_context(tc.tile_pool(name="wo", bufs=1))
        wo_sb = [wopool.tile([128, HID], F16, tag=f"wo{i}", name=f"wo{i}") for i in range(4)]

        qk_pool = persist.enter_context(tc.tile_pool(name="qk", bufs=1))
        qk_sb = [qk_pool.tile([128, S], F16, tag=f"qk{r}", name=f"qk{r}") for r in range(QH + KVH)]
        v_pool = persist.enter_context(tc.tile_pool(name="v", bufs=1))
        v_sb = [v_pool.tile([128, KVH * HD], F16, tag=f"v{t}", name=f"v{t}") for t in range(JT)]
        g_pool = persist.enter_context(tc.tile_pool(name="g", bufs=1))
        sgbc = [g_pool.tile([128, S], F16, tag=f"sg{h}", name=f"sg{h}") for h in range(QH)]

        # ---------------- phase 1: qkv projection + rope + v transpose -----
        with ExitStack() as ph1:
            xpool = ph1.enter_context(tc.tile_pool(name="x", bufs=32))
            tmppool = ph1.enter_context(tc.tile_pool(name="tmp", bufs=3))
            vrawpool = ph1.enter_context(tc.tile_pool(name="vraw", bufs=2))
            sgpool = ph1.enter_context(tc.tile_pool(name="sg", bufs=1))

            ps_acc = ph1.enter_context(tc.tile_pool(name="acc", bufs=4, space="PSUM"))
            ps_rot = ph1.enter_context(tc.tile_pool(name="rot", bufs=2, space="PSUM"))
            ps_tp = ph1.enter_context(tc.tile_pool(name="tp", bufs=2, space="PSUM"))

            pending_proc = [None]

            def emit_pending():
                if pending_proc[0] is not None:
                    pending_proc[0]()
                    pending_proc[0] = None

            def emit_gate(ib, xb, sl):
                psg_full = ps_rot.tile([128, IB], F32, tag="rot", name="psg")
                for h in range(16):
                    nc.tensor.matmul(psg_full, wsb2[h][:, WCOL - 640:WCOL - 512], xb[h],
                                     start=(h == 0), stop=(h == 15))
                emit_pending()
                psg = psg_full[96:96 + QH, :]
                eT = sgpool.tile([QH, IB], F32, tag="eT", name="eT")
                nc.scalar.activation(out=eT, in_=psg, func=AF.Exp, scale=-1.0)
                nc.vector.tensor_scalar_add(eT, eT, 1.0)
                sgT = sgpool.tile([QH, IB], F32, tag="sgT", name="sgT")
                nc.vector.reciprocal_approx_fast(out=sgT, in_=eT)
                sgT16 = sgpool.tile([QH, IB], F16, tag="sgT16", name="sgT16")
                nc.vector.tensor_copy(sgT16, sgT)
                nc.sync.dma_start(out=sgs_d[:, sl], in_=sgT16)
                for h in range(QH):
                    nc.scalar.dma_start(
                        out=sgbc[h][:, sl],
                        in_=sgs_d[h:h + 1, sl].to_broadcast((128, IB)))

            for ib in range(NB):
                sl = slice(IB * ib, IB * (ib + 1))
                xb = []
                for h in range(16):
                    xt = xpool.tile([128, IB], F16, tag="x", name="x")
                    # block 0 is latency-critical: use sync's hardware DGE for
                    # every tile (gpsimd's software DGE costs ~640ns/descriptor,
                    # staggering even-h tiles at startup). Later blocks
                    # prefetch with slack and spread across both engines.
                    if ib == 0:
                        eng = nc.sync
                    else:
                        eng = nc.gpsimd if h % 2 == 0 else nc.sync
                    eng.dma_start(
                        out=xt, in_=xT_d[HID * ib + 128 * h:HID * ib + 128 * (h + 1), :])
                    xb.append(xt)

                # stationary groups of output row-tiles. Block 0 uses two
                # groups of 4 (halves the chunk-walk rate so the startup
                # DMA supply keeps up); later blocks use 4 groups of 2
                # (software-pipelined procs).
                groups = ([[0, 1, 2, 3], [4, 5, 6, 7]] if ib == 0 else
                          [[0, 1], [2, 3], [4, 5], [6, 7]])
                for gi, rs in enumerate(groups):
                    accs = [ps_acc.tile([128, IB], F32, tag="acc", name="acc")
                            for _ in rs]
                    for h in range(16):
                        for r2, r in enumerate(rs):
                            wgrp = wsb1[h] if r < 4 else wsb2[h]
                            c0 = 128 * r if r < 4 else 128 * (r - 4)
                            nc.tensor.matmul(
                                accs[r2], wgrp[:, c0:c0 + 128], xb[h],
                                start=(h == 0), stop=(h == 15))

                    def make_proc(rs, accs, sl):
                        def proc():
                            for r2, r in enumerate(rs):
                                if r < QH + KVH:  # q or k row-tile: rope
                                    craw = tmppool.tile([128, IB], F16, tag="craw", name="craw")
                                    nc.scalar.copy(craw, accs[r2])
                                    rps = ps_rot.tile([128, IB], F32, tag="rot", name="rot")
                                    nc.tensor.matmul(rps, rotm_sb, craw, start=True, stop=True)
                                    t1 = tmppool.tile([128, IB], F32R, tag="t1", name="t1")
                                    nc.vector.tensor_mul(t1, accs[r2], cos_sb[:, sl])
                                    t2 = tmppool.tile([128, IB], F32R, tag="t2", name="t2")
                                    nc.vector.tensor_mul(t2, rps, sin_sb[:, sl])
                                    nc.vector.tensor_add(qk_sb[r][:, sl], t1, t2)
                                else:  # v row-tile: transpose to [tokens, d]
                                    vraw = vrawpool.tile([128, IB], F16, tag="vraw", name="vraw")
                                    nc.scalar.copy(vraw, accs[r2])
                                    vh = r - (QH + KVH)
                                    ibb = (sl.start // IB)
                                    for s2 in range(IB // 128):
                                        tp = ps_tp.tile([128, 128], F16, tag="tp", name="tp")
                                        nc.tensor.transpose(
                                            tp, vraw[:, 128 * s2:128 * (s2 + 1)], ident_sb)
                                        tt = (IB // 128) * ibb + s2
                                        nc.vector.tensor_copy(
                                            v_sb[tt][:, 128 * vh:128 * (vh + 1)], tp)
                        return proc

                    if ib == 0:
                        if gi == 0:
                            emit_gate(ib, xb, sl)
                        make_proc(rs, accs, sl)()
                    else:
                        emit_pending()
                        pending_proc[0] = make_proc(rs, accs, sl)

                if ib == 0:
                    continue
                emit_gate(ib, xb, sl)

            for cc in range(4):
                nc.gpsimd.dma_start(out=wo_sb[cc], in_=woT_d[128 * cc:128 * (cc + 1), :])
            emit_pending()

        # ---------------- phase 2: attention + gate + out-projection -------
        with ExitStack() as ph2:
            oc_pool = ph2.enter_context(tc.tile_pool(name="oc", bufs=1))
            OC = [oc_pool.tile([128, S], F16, tag=f"oc{h}", name=f"oc{h}") for h in range(QH)]
            epool = ph2.enter_context(tc.tile_pool(name="e", bufs=6))
            accpool = ph2.enter_context(tc.tile_pool(name="dacc", bufs=4))
            popool = ph2.enter_context(tc.tile_pool(name="po", bufs=4))
            scpool = ph2.enter_context(tc.tile_pool(name="sc", bufs=2))
            foldpool = ph2.enter_context(tc.tile_pool(name="fold", bufs=4))
            ypool = ph2.enter_context(tc.tile_pool(name="y", bufs=3))

            ps_s = ph2.enter_context(tc.tile_pool(name="pss", bufs=2, space="PSUM"))
            ps_o = ph2.enter_context(tc.tile_pool(name="pso", bufs=2, space="PSUM"))
            ps_sh = ph2.enter_context(tc.tile_pool(name="pssh", bufs=2, space="PSUM"))

            def oproj_steps(t, drain_all_dve):
                """out-projection for token tile t as 8 closures of ~2 MMs each."""
                state = {}

                def start():
                    state["ysb"] = ypool.tile([128, HID], F16, tag="y", name="y")

                steps = []
                for o in range(4):
                    def s_a(o=o):
                        if o == 0:
                            start()
                        state[o] = ps_sh.tile([128, IBLK], F32, tag="sh", name="psy")
                        for cc in range(2):
                            nc.tensor.matmul(
                                state[o], OC[cc][:, 128 * t:128 * (t + 1)],
                                wo_sb[cc][:, IBLK * o:IBLK * (o + 1)],
                                start=(cc == 0), stop=False)

                    def s_b(o=o):
                        for cc in range(2, 4):
                            nc.tensor.matmul(
                                state[o], OC[cc][:, 128 * t:128 * (t + 1)],
                                wo_sb[cc][:, IBLK * o:IBLK * (o + 1)],
                                start=False, stop=(cc == 3))
                        ysb = state["ysb"]
                        dst = ysb[:, IBLK * o:IBLK * (o + 1)]
                        if drain_all_dve:
                            nc.vector.tensor_copy(dst, state[o])
                        elif o % 2 == 0:
                            nc.scalar.copy(dst, state[o])
                        else:
                            nc.vector.tensor_copy(dst, state[o])
                        # per-o y DMA on alternating queues: starts output
                        # transfers early and spreads them off one DMA queue
                        eng = (nc.gpsimd, nc.sync, nc.scalar)[o % 3]
                        eng.dma_start(
                            out=y_d[128 * t:128 * (t + 1), IBLK * o:IBLK * (o + 1)],
                            in_=dst)

                    steps.append(s_a)
                    steps.append(s_b)
                return steps

            def emit_jloop(i, kv, fillers):
                """software-pipelined scores->exp->PV loop; fillers: list of
                closures (PE micro-steps) consumed one per pipeline slot."""
                isl = slice(IBLK * i, IBLK * (i + 1))
                psos = []
                accs2 = []
                for hh in range(2):
                    psos.append(ps_o.tile([128, IBLK], F32, tag="pso", name="pso"))
                    accs2.append(accpool.tile([128, 2 * IBLK], F16, tag="dacc", name="dacc"))
                fill_iter = iter(fillers)
                es_prev = None
                for jp in range(JP + 1):
                    es = None
                    if jp < JP:
                        es = []
                        for hh in range(2):
                            h = 2 * kv + hh
                            pss = ps_s.tile([128, 2 * IBLK], F32, tag="pss", name="pss")
                            for jj in range(2):
                                j = 2 * jp + jj
                                jsl = slice(128 * j, 128 * (j + 1))
                                nc.tensor.matmul(pss[:, IBLK * jj:IBLK * (jj + 1)],
                                                 qk_sb[QH + kv][:, jsl],
                                                 qk_sb[h][:, isl], start=True, stop=True)
                            if jp == 0:
                                e = accs2[hh]
                            else:
                                e = epool.tile([128, 2 * IBLK], F16, tag="e", name="e")
                            nc.scalar.activation(out=e, in_=pss, func=AF.Exp, scale=SCALE)
                            es.append(e)
                    if jp >= 1:
                        jq = jp - 1
                        # fillers first: they have no exp dependency, so they
                        # cover any exp latency before the PV matmuls
                        step = next(fill_iter, None)
                        if step is not None:
                            step()
                        for hh in range(2):
                            if jq > 0:
                                nc.vector.tensor_add(accs2[hh], accs2[hh], es_prev[hh])
                            for jj in range(2):
                                j = 2 * jq + jj
                                nc.tensor.matmul(psos[hh],
                                                 v_sb[j][:, 128 * kv:128 * (kv + 1)],
                                                 es_prev[hh][:, IBLK * jj:IBLK * (jj + 1)],
                                                 start=(j == 0), stop=(j == JT - 1))
                    es_prev = es
                # drain PV PSUM to SBUF fp16 (frees banks; decouples den chain)
                psout = []
                for hh in range(2):
                    po = popool.tile([128, IBLK], F16, tag="po", name="po")
                    nc.vector.tensor_copy(po, psos[hh])
                    psout.append(po)
                # fold the e-sum pairs early on DVE (feeds deferred den matmul)
                folds = []
                for hh in range(2):
                    fold = foldpool.tile([128, IBLK], F16, tag="fold", name="fold")
                    nc.vector.tensor_add(fold, accs2[hh][:, :IBLK], accs2[hh][:, IBLK:])
                    folds.append(fold)
                # leftover fillers (i==0 loops have none queued anyway)
                for step in fill_iter:
                    step()
                return psout, folds

            def emit_den_mms(kv, folds, dens):
                """den broadcast to all 128 rows via all-ones stationary."""
                for hh in range(2):
                    den = ps_sh.tile([128, IBLK], F32, tag="sh", name="den")
                    nc.tensor.matmul(den, onesN_sb, folds[hh], start=True, stop=True)
                    dens.append(den)

            def emit_sc_oc(i, kv, dens, psout):
                """recip + gate-scale + OC write, all DVE (no PE in chain)."""
                isl = slice(IBLK * i, IBLK * (i + 1))
                for hh in range(2):
                    h = 2 * kv + hh
                    rec = scpool.tile([128, IBLK], F32, tag="rec", name="rec")
                    nc.vector.reciprocal_approx_fast(out=rec, in_=dens[hh])
                    scg = scpool.tile([128, IBLK], F16, tag="scg", name="scg")
                    nc.vector.tensor_mul(scg, rec, sgbc[h][:, isl])
                    nc.vector.tensor_mul(OC[h][:, isl], psout[hh], scg)

            def emit_oproj_direct(t, cover=None):
                steps = oproj_steps(t, drain_all_dve=False)
                out = []
                for idx, step in enumerate(steps):
                    step()
                    if cover is not None and idx == 3:
                        cover()
                return out

            # emission schedule
            for i in range(NI):
                f0 = oproj_steps(4 * (i - 1) + 2, True) if i > 0 else []
                po0, fo0 = emit_jloop(i, 0, f0)
                f1 = oproj_steps(4 * (i - 1) + 3, True) if i > 0 else []
                po1, fo1 = emit_jloop(i, 1, f1)
                dens0 = []
                emit_den_mms(0, fo0, dens0)
                emit_sc_oc(i, 0, dens0, po0)
                if i > 0:
                    emit_oproj_direct(4 * (i - 1) + 0)
                dens1 = []
                emit_den_mms(1, fo1, dens1)
                emit_sc_oc(i, 1, dens1, po1)
                if i > 0:
                    emit_oproj_direct(4 * (i - 1) + 1)
            for t in (12, 13, 14, 15):
                emit_oproj_direct(t)

    nc.finalize()
    return nc


def kernel(hidden_states, cos, sin, w_qkv, w_o):
    global LAST_EXEC_NS, LAST_RESULTS
    from concourse.bass_utils import run_bass_kernel_spmd

    BF = np.float16
    hidden_states = np.asarray(hidden_states, dtype=np.float32)
    cos = np.asarray(cos, dtype=np.float32)
    sin = np.asarray(sin, dtype=np.float32)
    w_qkv = np.asarray(w_qkv, dtype=np.float32)
    w_o = np.asarray(w_o, dtype=np.float32)

    if "nc" not in _CACHE:
        _CACHE["nc"] = _build_program()
    nc = _CACHE["nc"]

    cosT = np.ascontiguousarray(cos.T).astype(BF)
    sinT = np.ascontiguousarray(sin.T).astype(BF)
    rotm = np.zeros((HD, HD), dtype=np.float32)
    for i in range(HD // 2):
        rotm[i + HD // 2, i] = -1.0   # rot[d'] = -q[d'+64] for d' < 64
        rotm[i, i + HD // 2] = 1.0    # rot[d'] = +q[d'-64] for d' >= 64
    rotm = rotm.astype(BF)
    ident = np.eye(128, dtype=np.float32).astype(BF)
    onesr = np.ones((1, 128), dtype=np.float32)
    onesN = np.ones((128, 128), dtype=np.float16)

    # pre-tiled x: [NB*HID, IB], block-major, sequential HBM reads per tile
    xT = []
    for b in range(B):
        xb_ = hidden_states[b].T.reshape(HID, NB, IB).transpose(1, 0, 2)
        xT.append(np.ascontiguousarray(xb_.reshape(NB * HID, IB)).astype(BF))
    in_maps = []
    for c in range(N_CORES):
        b, g = divmod(c, TPG)
        qr = w_qkv[512 * g:512 * (g + 1)]
        kr = w_qkv[HID + GATE + 256 * g:HID + GATE + 256 * (g + 1)]
        vr = w_qkv[HID + GATE + KV_DIM + 256 * g:HID + GATE + KV_DIM + 256 * (g + 1)]
        gr = w_qkv[HID + QH * g:HID + QH * (g + 1)]
        pad = np.zeros((WCOL - 1024 - QH, HID), dtype=np.float32)
        wqkvT = np.ascontiguousarray(
            np.concatenate([qr, kr, vr, gr, pad], axis=0).T).astype(BF)
        wqkv1 = np.ascontiguousarray(wqkvT[:, :512])
        wqkv2 = np.ascontiguousarray(wqkvT[:, 512:])
        woT = np.ascontiguousarray(w_o[:, 512 * g:512 * (g + 1)].T).astype(BF)
        in_maps.append({
            "xT": xT[b], "wqkv1": wqkv1, "wqkv2": wqkv2, "woT": woT,
            "cosT": cosT, "sinT": sinT, "rotm": rotm, "ident": ident,
            "onesr": onesr, "onesN": onesN,
        })

    trace = bool(int(os.environ.get("KERNEL_TRACE", "0")))
    out = run_bass_kernel_spmd(nc, in_maps, list(range(N_CORES)), trace=trace)
    LAST_EXEC_NS = out.exec_time_ns
    LAST_RESULTS = out
    y = np.zeros((B, S, HID), dtype=np.float32)
    for c in range(N_CORES):
        b = c // TPG
        y[b] += np.asarray(out.results[c]["y"]).astype(np.float32)
    return y
